# revision 1
# baseline (speedup 1.0000x reference)
"""Trainium2 Bass kernel for CapsuleLayer (nn_CapsuleLayer_45552423142009).

Computes, for x[B,768]:
  u = squash(x @ Wp + bp)            # [B, 8, 16]  (squash over last dim)
  u_hat[b,p,c,:] = u[b,p,:] @ W[p,c] # [B, 8, 5, 16]
  3 iterations of dynamic routing -> v [B, 5, 16]

Strategy: pure data-parallel over 8 NeuronCores (batch sharded 16384/core).
On-chip layout is "transposed": features on partitions, batch on the free
dim (512-wide tiles).  The host pre-transposes x so tiles DMA directly as
[d, b] (no PE transposes), and the output is written feature-major
[16j, 5c, b] (host transposes back).  PE does the two big matmuls and all
broadcast / segment-sum reductions (0/1 selector matrices, fp32r at
1 cycle/row).  Squash factors use exp(0.5*ln(sq) - ln(1+sq)) so the ACT
engine stays on the single natural_log_exp table (no table thrash);
softmax reciprocals use the fast custom-DVE approximation.
"""

import sys
import numpy as np

sys.path.insert(0, "/opt/trn_rl_repo")

from concourse import bass, bacc, mybir  # noqa: E402
from concourse import tile  # noqa: E402
from concourse.bass_utils import run_bass_kernel_spmd  # noqa: E402
from concourse.alu_op_type import AluOpType  # noqa: E402

F32 = mybir.dt.float32
F32R = mybir.dt.float32r
AF = mybir.ActivationFunctionType

B = 131072
D = 768
P = 8
PD = 16
C = 5
CD = 16
NCORES = 8
BC = B // NCORES          # 16384 batch rows per core
NB = 512                  # batch columns per tile
NT = BC // NB             # 32 tiles

# selector blob column offsets (blob layout shared with the host packer)
SEL_SSEL8 = 0      # [128, 8]   sum o-groups of 16 -> p
SEL_PSEL16 = 8     # [128, 16]  0.2 * (sum over p at fixed j)
SEL_PSEL8 = 24     # [128, 8]   sum over j at fixed p (unused)
SEL_IDENT = 32     # [128, 128] identity (unused)
SEL_TILE8 = 160    # [16, 128]  broadcast j -> (p, j)
SEL_SBC = 288      # [8, 128]   broadcast p -> (p, o)
SEL_JSEL = 416     # [80, 5]    (unused)
SEL_JBC = 421      # [5, 80]    (unused)
SEL_CSEL = 501     # [40, 8]    sum over c at fixed p   (logits layout (c,p))
SEL_CBC = 509      # [8, 40]    broadcast p -> (c, p)
SEL_BSEL = 549     # [40, 640]  5 x [40,128]: broadcast (c,p) -> (p,i) for class c
SEL_ESEL = 1190    # [16, 40]   5 x [16,8]: col c ones (vsq accumulate)
SEL_GBC = 1230     # [8, 80]    5 x [8,16]: row c ones (g -> j-bcast, class c)
SEL_GBC40 = 1310   # [8, 40]    [c', (c,p)] = d_c'c (g -> (c,p) bcast)
SEL_ASEL = 1350    # [128, 200] 5 x [128,40]: [(p,j),(c',p')] = d_pp' d_c'c
SEL_T80 = 1550     # [80, 640]  5 x [80,128]: [(c',j'),(p,j)] = d_c'c d_j'j
SEL_W = 2190
OFF_WP = SEL_W                 # [128, 768]
OFF_WBD = OFF_WP + 768         # [128, 640]
OFF_WF02 = OFF_WBD + 640       # [128, 80]  0.2 * wflat  (itr0 s in one matmul)
OFF_WFM = OFF_WF02 + 80        # [128, 400] 5 x class-masked wflat
OFF_BP = OFF_WFM + 400         # [128, 1]
CST_W = OFF_BP + 1


def _r(ap):
    return ap.bitcast(F32R)


class _BaccOneActTable(bacc.Bacc):
    """Bacc that pins every activation to the natural_log_exp table.

    The default table chooser picks the first act_info table containing
    each function (Ln -> natural_log, Exp -> exp_and_others), which
    thrashes ACT_TABLE_LOADs at every Ln/Exp boundary.  All functions this
    kernel uses (Ln, Exp, Square, Identity, Copy) live together in
    natural_log_exp_and_others, so present that as the only non-empty
    table while keeping list indices canonical (act_func_set_id is an
    index into act_info.json's array).
    """

    _TABLE = "natural_log_exp_and_others"

    def insert_act_table_loads(self):
        import bass_rust as _bass_rust
        from concourse.hw_specs import get_activation_tables

        has_activation = any(
            isinstance(i, mybir.InstActivation)
            for b in self.main_func.blocks
            for i in b.instructions
        )
        if not has_activation:
            return
        tables = [
            (name, funcs if name == self._TABLE else set())
            for name, funcs in get_activation_tables(self.m.arch).items()
        ]
        _bass_rust.insert_act_table_loads(self, tables)


def build_selectors() -> np.ndarray:
    sel = np.zeros((128, SEL_W), dtype=np.float32)
    for m in range(128):
        sel[m, SEL_SSEL8 + m // 16] = 1.0                      # Ssel8
    for p in range(P):
        for j in range(PD):
            sel[p * 16 + j, SEL_PSEL16 + j] = 0.2              # Psel16 (x0.2)
            sel[p * 16 + j, SEL_PSEL8 + p] = 1.0               # Psel8
    sel[:, SEL_IDENT:SEL_IDENT + 128] = np.eye(128, dtype=np.float32)
    for j in range(16):
        for p in range(P):
            sel[j, SEL_TILE8 + p * 16 + j] = 1.0               # Tile8
    for p in range(P):
        sel[p, SEL_SBC + p * 16:SEL_SBC + (p + 1) * 16] = 1.0  # Sbc
    for c in range(C):
        for j in range(CD):
            sel[c * 16 + j, SEL_JSEL + c] = 1.0                # Jsel
            sel[c, SEL_JBC + c * 16 + j] = 1.0                 # Jbc
    for c in range(C):
        for p in range(P):
            sel[c * 8 + p, SEL_CSEL + p] = 1.0                 # Csel
            sel[p, SEL_CBC + c * 8 + p] = 1.0                  # Cbc
            # Bsel_c: [(c',p), (p',i)] = d_cc' d_pp'
            sel[c * 8 + p, SEL_BSEL + c * 128 + p * 16:
                SEL_BSEL + c * 128 + (p + 1) * 16] = 1.0
    for c in range(C):
        for j in range(CD):
            sel[j, SEL_ESEL + c * 8 + c] = 1.0                 # Esel_c col c
            sel[c, SEL_GBC + c * 16 + j] = 1.0                 # Gbc_c row c
    for c in range(C):
        for p in range(P):
            sel[c, SEL_GBC40 + c * 8 + p] = 1.0                # Gbc40
    for c in range(C):
        for p in range(P):
            for j in range(CD):
                # Asel_c: [(p,j), (c',p')] = d_pp' d_c'c
                sel[p * 16 + j, SEL_ASEL + c * 40 + c * 8 + p] = 1.0
    for c in range(C):
        for p in range(P):
            for j in range(CD):
                # Tile8_80_c: [(c',j'), (p,j)] = d_c'c d_j'j
                sel[c * 16 + j, SEL_T80 + c * 128 + p * 16 + j] = 1.0
    return sel


def build_nc(nt: int = NT) -> bass.Bass:
    bc = nt * NB
    nc = _BaccOneActTable(None)

    x_d = nc.declare_dram_parameter("xt", [D, bc], F32R, isOutput=False)
    cst_d = nc.declare_dram_parameter("cst", [128, CST_W], F32R, isOutput=False)
    v_d = nc.declare_dram_parameter("vout", [C * CD, bc], F32, isOutput=True)

    with tile.TileContext(nc) as tc, nc.allow_low_precision(reason="float32r matmul inputs"):
        with (
            tc.sbuf_pool(name="const", bufs=1) as cpool,
            tc.sbuf_pool(name="xt", bufs=3) as xtpool,
            tc.sbuf_pool(name="mid", bufs=3) as mpool,
            tc.sbuf_pool(name="uh", bufs=2) as uhpool,
            tc.sbuf_pool(name="rt", bufs=2) as rtpool,
            tc.sbuf_pool(name="sm", bufs=6) as smpool,
            tc.psum_pool(name="pmm", bufs=2) as pmmp,
            tc.psum_pool(name="pbc", bufs=3) as pbcp,
            tc.psum_pool(name="psc", bufs=2) as pscp,
            tc.psum_pool(name="psm", bufs=1) as psmp,
        ):
            # ---- load constants (one DMA), then stage through DVE so every
            # consumer depends on the DVE semaphore ----
            cst0 = cpool.tile([128, CST_W], F32R)
            nc.sync.dma_start(out=cst0[:], in_=cst_d[:])
            cst = cpool.tile([128, CST_W], F32R)
            nc.vector.tensor_copy(cst[:], cst0[:])
            sel_sb = cst[:, 0:SEL_W]
            wp_sb = cst[:, OFF_WP:OFF_WP + 768]
            wbd_sb = cst[:, OFF_WBD:OFF_WBD + 640]
            wf02_sb = cst[:, OFF_WF02:OFF_WF02 + 80]
            wfm_sb = cst[:, OFF_WFM:OFF_WFM + 400]
            bp_sb = cst[:, OFF_BP:OFF_BP + 1].bitcast(F32)

            class TS:
                """Per-tile routing state."""
                def __init__(self, it):
                    self.it = it
                    self.logit = None

            def p_load(s):
                s.xT = xtpool.tile([128, 6, NB], F32R, tag="xt")
                src = x_d[:, s.it * NB:(s.it + 1) * NB].rearrange(
                    "(k p) b -> p k b", p=128)
                nc.sync.dma_start(out=s.xT[:], in_=src)

            def p_mm1(s):
                # u_pre[(p,o), b] = Wp^T xT (+bias); usq = (pre)^2 from PSUM
                s.pu = pmmp.tile([128, NB], F32, tag="pmm")
                for k in range(6):
                    nc.tensor.matmul(
                        s.pu[:], _r(wp_sb[:, k * 128:(k + 1) * 128]),
                        _r(s.xT[:, k, :]), start=(k == 0), stop=(k == 5))
                s.u_pre = mpool.tile([128, NB], F32, tag="mid")
                nc.scalar.activation(s.u_pre[:], s.pu[:], AF.Identity,
                                     bias=bp_sb[:], scale=1.0)
                s.usq = mpool.tile([128, NB], F32R, tag="mid2")
                nc.scalar.activation(s.usq[:], s.pu[:], AF.Square,
                                     bias=bp_sb[:], scale=1.0)

            def p_psq(s):
                # f[p, b] = exp(0.5*ln(sq) - ln(1+sq)) chain kicked off here
                s.psq = psmp.tile([8, NB], F32, tag="psm")
                nc.tensor.matmul(
                    s.psq[:], _r(sel_sb[:, SEL_SSEL8:SEL_SSEL8 + 8]),
                    _r(s.usq[:]), start=True, stop=True)
                w_ln = smpool.tile([8, NB], F32, tag="sm")
                nc.scalar.activation(w_ln[:], s.psq[:], AF.Ln)
                ln1 = smpool.tile([8, NB], F32, tag="sm")
                nc.scalar.activation(ln1[:], s.psq[:], AF.Ln,
                                     bias=1.0, scale=1.0)
                zf = smpool.tile([8, NB], F32, tag="sm")
                nc.vector.scalar_tensor_tensor(
                    zf[:], w_ln[:], 0.5, ln1[:],
                    op0=AluOpType.mult, op1=AluOpType.subtract)
                s.fz = smpool.tile([8, NB], F32R, tag="sm")
                nc.scalar.activation(s.fz[:], zf[:], AF.Exp)

            def p_uh(s):
                pfb = pbcp.tile([128, NB], F32, tag="pbc")
                nc.tensor.matmul(
                    pfb[:], _r(sel_sb[:8, SEL_SBC:SEL_SBC + 128]),
                    _r(s.fz[:]), start=True, stop=True)
                s.u = mpool.tile([128, NB], F32R, tag="mid3")
                nc.vector.tensor_mul(s.u[:], s.u_pre[:], pfb[:])
                s.uh = uhpool.tile([128, C, NB], F32R, tag="uh")
                for c in range(C):
                    puh = pmmp.tile([128, NB], F32, tag="pmm")
                    nc.tensor.matmul(
                        puh[:], _r(wbd_sb[:, c * 128:(c + 1) * 128]),
                        _r(s.u[:]), start=True, stop=True)
                    if c % 2 == 0:
                        nc.scalar.copy(s.uh[:, c, :], puh[:])
                    else:
                        nc.vector.tensor_copy(s.uh[:, c, :], puh[:])

            def p_soft_head(s):
                # softmax numerator + denominator head (up to rdr)
                s.e = rtpool.tile([40, NB], F32R, tag="rt_e")
                nc.scalar.activation(s.e[:], s.logit[:], AF.Exp)
                s.pden = psmp.tile([8, NB], F32, tag="psm")
                nc.tensor.matmul(
                    s.pden[:], _r(sel_sb[:40, SEL_CSEL:SEL_CSEL + 8]),
                    _r(s.e[:]), start=True, stop=True)
                rd = smpool.tile([8, NB], F32, tag="sm")
                nc.vector.reciprocal_approx_fast(out=rd[:], in_=s.pden[:])
                s.rdr = smpool.tile([8, NB], F32R, tag="sm")
                nc.vector.tensor_copy(s.rdr[:], rd[:])

            def p_soft_tail(s):
                pdb = pbcp.tile([40, NB], F32, tag="pbc")
                nc.tensor.matmul(
                    pdb[:], _r(sel_sb[:8, SEL_CBC:SEL_CBC + 40]),
                    _r(s.rdr[:]), start=True, stop=True)
                s.cn = rtpool.tile([40, NB], F32R, tag="rt_cn")
                nc.vector.tensor_mul(s.cn[:], s.e[:], pdb[:])

            def p_s_bc(s):
                # all 5 class broadcasts back-to-back; t-muls trail on DVE
                s.ts = []
                for c in range(C):
                    pcb = pbcp.tile([128, NB], F32, tag="pbc")
                    nc.tensor.matmul(
                        pcb[:], _r(sel_sb[:40, SEL_BSEL + c * 128:
                                          SEL_BSEL + (c + 1) * 128]),
                        _r(s.cn[:]), start=True, stop=True)
                    t = rtpool.tile([128, NB], F32R, tag=f"rt_t{c}")
                    nc.vector.tensor_mul(t[:], s.u[:], pcb[:])
                    s.ts.append(t)

            def p_s_mm(s, itr):
                # psc80[(c,j), b]: itr0 one matmul 0.2*wflat^T u;
                # itr>0 five class-masked accumulating matmuls on t_c
                s.psc = pscp.tile([80, NB], F32, tag="psc")
                if itr == 0:
                    nc.tensor.matmul(s.psc[:], _r(wf02_sb[:]), _r(s.u[:]),
                                     start=True, stop=True)
                else:
                    for c in range(C):
                        nc.tensor.matmul(
                            s.psc[:], _r(wfm_sb[:, c * 80:(c + 1) * 80]),
                            _r(s.ts[c][:]), start=(c == 0), stop=(c == 4))

            def p_s_tail(s):
                # copy + square read the [80, NB] PSUM in parallel
                s.s80 = rtpool.tile([80, NB], F32R, tag="rt_s")
                nc.vector.tensor_copy(s.s80[:], s.psc[:])
                s.ssq = rtpool.tile([80, NB], F32R, tag="rt_ssq")
                nc.scalar.activation(s.ssq[:], s.psc[:], AF.Square)

            def p_vsq(s):
                s.pvq = psmp.tile([5, NB], F32, tag="psm")
                nc.tensor.matmul(
                    s.pvq[:], _r(sel_sb[:80, SEL_JSEL:SEL_JSEL + 5]),
                    _r(s.ssq[:]), start=True, stop=True)

            def p_g(s):
                # g = exp(0.5*ln(vsq) - ln(1+vsq))
                wg = smpool.tile([5, NB], F32, tag="sm")
                nc.scalar.activation(wg[:], s.pvq[:], AF.Ln)
                lg1 = smpool.tile([5, NB], F32, tag="sm")
                nc.scalar.activation(lg1[:], s.pvq[:], AF.Ln,
                                     bias=1.0, scale=1.0)
                zg = smpool.tile([5, NB], F32, tag="sm")
                nc.vector.scalar_tensor_tensor(
                    zg[:], wg[:], 0.5, lg1[:],
                    op0=AluOpType.mult, op1=AluOpType.subtract)
                s.g = smpool.tile([5, NB], F32R, tag="sm")
                nc.scalar.activation(s.g[:], zg[:], AF.Exp)

            def p_ag_bc(s):
                s.prs = []
                for c in range(C):
                    pvb = pbcp.tile([128, NB], F32, tag="pbc")
                    nc.tensor.matmul(
                        pvb[:], _r(sel_sb[:80, SEL_T80 + c * 128:
                                          SEL_T80 + (c + 1) * 128]),
                        _r(s.s80[:]), start=True, stop=True)
                    pr = rtpool.tile([128, NB], F32R, tag=f"rt_pr{c}")
                    nc.vector.tensor_mul(pr[:], s.uh[:, c, :], pvb[:])
                    s.prs.append(pr)

            def p_ag_mm(s):
                for c in range(C):
                    nc.tensor.matmul(
                        s.pat[:], _r(sel_sb[:, SEL_ASEL + c * 40:
                                            SEL_ASEL + (c + 1) * 40]),
                        _r(s.prs[c][:]), start=(c == 0), stop=(c == 4))

            def p_ag_tail(s, itr):
                ats = rtpool.tile([40, NB], F32, tag="rt_ats")
                nc.scalar.copy(ats[:], s.pat[:])
                pg40 = pbcp.tile([40, NB], F32, tag="pbc")
                nc.tensor.matmul(
                    pg40[:], _r(sel_sb[:5, SEL_GBC40:SEL_GBC40 + 40]),
                    _r(s.g[:]), start=True, stop=True)
                if itr == 0:
                    s.logit = rtpool.tile([40, NB], F32, tag="rt_lg")
                    nc.vector.tensor_mul(s.logit[:], ats[:], pg40[:])
                else:
                    a40 = rtpool.tile([40, NB], F32, tag="rt_a40")
                    nc.vector.tensor_mul(a40[:], ats[:], pg40[:])
                    lg2 = rtpool.tile([40, NB], F32, tag="rt_lg2")
                    nc.vector.tensor_add(lg2[:], s.logit[:], a40[:])
                    s.logit = lg2

            def p_fin(s):
                pgb = pbcp.tile([80, NB], F32, tag="pbc")
                nc.tensor.matmul(
                    pgb[:], _r(sel_sb[:5, SEL_JBC:SEL_JBC + 80]),
                    _r(s.g[:]), start=True, stop=True)
                s.v5 = rtpool.tile([80, NB], F32, tag="rt_v")
                nc.vector.tensor_mul(s.v5[:], s.s80[:], pgb[:])
                nc.sync.dma_start(
                    out=v_d[:, s.it * NB:(s.it + 1) * NB], in_=s.v5[:])

            # ---- software-pipelined pairs: twin tile's PE work covers the
            # serial ACT/DVE squash and softmax chains; the next pair's
            # head (load + mm1 + psq) is hoisted before this pair's tail so
            # PE stays fed across the pair boundary ----
            assert nt % 2 == 0 or nt == 1
            pairs = ([(TS(2 * i), TS(2 * i + 1)) for i in range(nt // 2)]
                     if nt > 1 else [(TS(0),)])

            def p_head(grp):
                for s in grp:
                    p_load(s)
                for s in grp:
                    p_mm1(s)
                for s in grp:
                    p_psq(s)

            def p_head(grp):
                for s in grp:
                    p_load(s)
                for s in grp:
                    p_mm1(s)
                for s in grp:
                    p_psq(s)

            p_head(pairs[0])
            for k, grp in enumerate(pairs):
                for s in grp:
                    p_uh(s)
                for itr in range(3):
                    if itr > 0:
                        for s in grp:
                            p_soft_head(s)
                        for s in grp:
                            p_soft_tail(s)
                    if itr > 0:
                        for s in grp:
                            p_s_bc(s)
                    for s in grp:
                        p_s_mm(s, itr)
                    if itr == 2 and k + 1 < len(pairs):
                        p_head(pairs[k + 1])
                    for s in grp:
                        p_s_tail(s)
                    for s in grp:
                        p_vsq(s)
                    for s in grp:
                        p_g(s)
                    if itr < 2:
                        for s in grp:
                            s.pat = pscp.tile([40, NB], F32, tag="psc")
                        for s in grp:
                            p_ag_bc(s)
                        for s in grp:
                            p_ag_mm(s)
                        for s in grp:
                            p_ag_tail(s, itr)
                for s in grp:
                    p_fin(s)

    nc.compile()
    return nc


_NC_CACHE: dict = {}


def _get_nc(nt: int) -> bass.Bass:
    if nt not in _NC_CACHE:
        _NC_CACHE[nt] = build_nc(nt)
    return _NC_CACHE[nt]


def _prep_weights(Wp, bp, W):
    Wp = np.asarray(Wp, np.float32)
    bp = np.asarray(bp, np.float32)
    W = np.asarray(W, np.float32)
    wp_flat = Wp.transpose(1, 0, 2).reshape(768, 128)          # [d, (p,o)]
    wp_h = np.ascontiguousarray(
        wp_flat.reshape(6, 128, 128).transpose(1, 0, 2).reshape(128, 768))
    wbd_h = np.zeros((128, 5, 128), np.float32)
    for p in range(P):
        wbd_h[p * 16:(p + 1) * 16, :, p * 16:(p + 1) * 16] = \
            W[p].transpose(1, 0, 2)                            # [i, c, j]
    wbd_h = np.ascontiguousarray(wbd_h.reshape(128, 640))
    wflat_h = np.ascontiguousarray(
        W.transpose(0, 2, 1, 3).reshape(128, 5 * 16))          # [(p,i), (c,j)]
    wf02_h = np.ascontiguousarray(0.2 * wflat_h)
    wfm_h = np.zeros((128, 5, 80), np.float32)
    for c in range(C):
        wfm_h[:, c, c * 16:(c + 1) * 16] = wflat_h[:, c * 16:(c + 1) * 16]
    wfm_h = np.ascontiguousarray(wfm_h.reshape(128, 400))
    bp_h = np.ascontiguousarray(bp.reshape(128, 1))
    sel_h = build_selectors()
    return wp_h, wbd_h, wflat_h, wf02_h, wfm_h, bp_h, sel_h


def pack_consts(Wp, bp, W):
    wp_h, wbd_h, wflat_h, wf02_h, wfm_h, bp_h, sel_h = _prep_weights(Wp, bp, W)
    cst = np.concatenate([sel_h, wp_h, wbd_h, wf02_h, wfm_h, bp_h], axis=1)
    assert cst.shape == (128, CST_W), cst.shape
    return np.ascontiguousarray(cst)


def make_in_maps(inputs, nt=NT):
    x = np.asarray(inputs["x"], np.float32)
    cst = pack_consts(inputs["Wp"], inputs["bp"], inputs["W"])
    bc = nt * NB
    return [{"xt": np.ascontiguousarray(x[i * BC:i * BC + bc].T), "cst": cst}
            for i in range(NCORES)]


def collect_out(res, nt=NT):
    # vout is [(c,j) = 80, bc] feature-major; transpose back to [b, c, j]
    outs = [res.results[i]["vout"].reshape(C, CD, -1).transpose(2, 0, 1)
            for i in range(NCORES)]
    return np.ascontiguousarray(np.concatenate(outs, axis=0))


def kernel(x, Wp, bp, W):
    nc = _get_nc(NT)
    in_maps = make_in_maps({"x": x, "Wp": Wp, "bp": bp, "W": W})
    res = run_bass_kernel_spmd(nc, in_maps, list(range(NCORES)))
    return collect_out(res)

